# revision 5
# baseline (speedup 1.0000x reference)
"""Locally-connected 2D block layer (LocBlock2dNT) on 8 Trainium2 NeuronCores.

Problem: x (64,64,64,64) f32, w (256,64,16,16,16) f32.
  patches = unfold(x) -> (N,C,P,P,f2);  y = relu(einsum('ncpqf,ocpqf->nopq', patches, w) / 32)

Strategy (v3, fp8 weights + fp8 activations, balanced DMA rings):
  - Shard over patch ROWS p (16 rows, 2 per core). Both x and w shard cleanly
    along p: zero replication.
  - Host-side (free): unfold + transpose into a K-major layout. w and x are
    cast to fp8 e3m4 (4 mantissa bits; ~1.9% end-to-end rel err vs the 2e-2
    budget) WITHOUT the 1/32 scale (values must stay in e3m4's normal range);
    the scale is applied in the epilogue: relu(ps/32) = max(ps*1/32, 0) as a
    two-op DVE tensor_scalar.
  - DMA per core: w 8.4 MB + x 2.1 MB in, y 1.05 MB out. Inputs stream in 8
    chunks alternating across the two HWDGE rings (sync/scalar) by parity so
    both rings carry equal bytes; outputs ride both rings after all inputs
    (never blocking an input DMA behind a compute dependency).
  - Per core: 32 positions, each an [M=64 batch] x [K=1024] x [N=256 outch]
    matmul (x stationary, w moving, both fp8e3). Positions packed
    two-at-a-time into the 128-wide PE column dim (pos A -> PSUM partitions
    0:64, pos B -> 64:128) so the two N=256 matmul streams run concurrently.
  - Epilogue: scale+relu on DVE, PSUM -> SBUF -> DRAM.
"""

import numpy as np
import ml_dtypes

N = 64          # batch
C = 64          # in channels
P = 16          # patches per side
F = 4           # filter side
F2 = F * F      # 16
O = 256         # out channels
K = C * F2      # 1024 contraction
NCORES = 8
PROWS_PER_CORE = P // NCORES      # 2
POS = PROWS_PER_CORE * P          # 32 positions per core
PAIRS = POS // 2                  # 16
KT = K // 128                     # 8 k-tiles
SCALE = 1.0 / np.sqrt(np.float32(F2 * C))   # == 1/32 exactly

BF16 = ml_dtypes.bfloat16
FP8 = ml_dtypes.float8_e3m4

X_FP8 = True                # x in fp8 e3m4 (else bf16 with scale folded in)

GP = 2                      # positions per chunk
NCHUNK = POS // GP          # 8

_cache = {}


def _build_program():
    """Build + compile the (SPMD, shared) Bass program once per process."""
    if "nc" in _cache:
        return _cache["nc"]

    import concourse.bacc as bacc
    import concourse.mybir as mybir
    import concourse.tile as tile

    xdt = mybir.dt.float8e3 if X_FP8 else mybir.dt.bfloat16

    nc = bacc.Bacc(
        "TRN2", target_bir_lowering=False, debug=False, num_devices=NCORES
    )
    xr = nc.dram_tensor("xr", (128, POS * KT * N), xdt,
                        kind="ExternalInput").ap()
    wr = nc.dram_tensor("wr", (128, POS * KT * O), mybir.dt.float8e3,
                        kind="ExternalInput").ap()
    # yr[r, pair*256 + o], r = (pos%2)*64 + n
    yr = nc.dram_tensor("yr", (128, PAIRS * O), mybir.dt.bfloat16,
                        kind="ExternalOutput").ap()

    XC = GP * KT * N            # x elems per partition per chunk
    WC = GP * KT * O            # w elems per partition per chunk

    with tile.TileContext(nc) as tc:
        QS = [nc.sync, nc.scalar]   # the two HWDGE rings
        with (
            tc.tile_pool(name="xpool", bufs=1) as xpool,
            tc.tile_pool(name="wpool", bufs=1) as wpool,
            tc.tile_pool(name="pspool", bufs=4, space="PSUM") as pspool,
            tc.tile_pool(name="opool", bufs=3) as opool,
        ):
            # whole x + w resident in SBUF, streamed in chunk-sized slices in
            # consumption order, alternating rings so both carry equal bytes.
            xall = xpool.tile([128, POS * KT * N], xdt)
            wall = wpool.tile([128, POS * KT * O], mybir.dt.float8e3)
            for c in range(NCHUNK):
                QS[c % 2].dma_start(out=wall[:, c * WC:(c + 1) * WC],
                                    in_=wr[:, c * WC:(c + 1) * WC])
                QS[(c + 1) % 2].dma_start(out=xall[:, c * XC:(c + 1) * XC],
                                          in_=xr[:, c * XC:(c + 1) * XC])

            for chunk in range(NCHUNK):
                ot = opool.tile([128, (GP // 2) * O], mybir.dt.bfloat16)
                for jp in range(GP // 2):      # position pairs in chunk
                    pos_a = chunk * GP + 2 * jp
                    pos_b = pos_a + 1
                    # two PSUM banks so the two concurrent accumulation
                    # groups never share a zero region
                    psa = pspool.tile([N, O], mybir.dt.float32)
                    psb_full = pspool.tile([128, O], mybir.dt.float32)
                    psb = psb_full[N:2 * N, :]
                    for k in range(KT):
                        xa = xall[:, pos_a * KT * N + k * N:
                                     pos_a * KT * N + k * N + N]
                        xb = xall[:, pos_b * KT * N + k * N:
                                     pos_b * KT * N + k * N + N]
                        wa = wall[:, pos_a * KT * O + k * O:
                                     pos_a * KT * O + k * O + O]
                        wb = wall[:, pos_b * KT * O + k * O:
                                     pos_b * KT * O + k * O + O]
                        # A -> array col group 0:64, B -> 64:128; the two
                        # matmul streams run concurrently
                        nc.tensor.matmul(psa, xa, wa,
                                         start=(k == 0), stop=(k == KT - 1))
                        nc.tensor.matmul(psb, xb, wb,
                                         start=(k == 0), stop=(k == KT - 1))
                    oc = jp * O
                    if X_FP8:
                        # relu(ps/32) == max(ps * 1/32, 0)
                        nc.vector.tensor_scalar(
                            ot[0:N, oc:oc + O], psa, float(SCALE), 0.0,
                            mybir.AluOpType.mult, mybir.AluOpType.max)
                        nc.vector.tensor_scalar(
                            ot[N:2 * N, oc:oc + O], psb, float(SCALE), 0.0,
                            mybir.AluOpType.mult, mybir.AluOpType.max)
                    else:
                        nc.vector.tensor_scalar_max(ot[0:N, oc:oc + O],
                                                    psa, 0.0)
                        nc.vector.tensor_scalar_max(ot[N:2 * N, oc:oc + O],
                                                    psb, 0.0)
                # one output DMA per chunk, alternating rings, issued after
                # all input DMAs in each ring's FIFO order
                pair0 = chunk * GP // 2
                QS[chunk % 2].dma_start(
                    out=yr[:, pair0 * O:(pair0 + GP // 2) * O], in_=ot)

    nc.compile()
    _cache["nc"] = nc
    return nc


def _prep_inputs(x: np.ndarray, w: np.ndarray):
    """Host-side shard + layout + cast. Returns in_maps for 8 cores.

    Layouts per core (core c owns patch rows 2c, 2c+1; pos = pl*16 + q):
      xr[p128, pos, k, n] = patches[n, ch, 2c+pl, q, f],  K = k*128+p128 = ch*16+f
      wr[p128, pos, k, o] = w[o, ch, 2c+pl, q, f]
      yr row = pair*128 + (pos%2)*64 + n
    Both fp8 e3m4 unscaled; the 1/32 scale is applied on-chip in the epilogue.
    """
    # unfold: (N,C,P,f,P,f) -> (N,C,P,P,f,f) -> (N,C,P,P,f2)
    patches = np.ascontiguousarray(
        x.reshape(N, C, P, F, P, F).transpose(0, 1, 2, 4, 3, 5)
    ).reshape(N, C, P, P, F2)
    if not X_FP8:
        patches = patches * SCALE
    ws = w.astype(np.float32)

    in_maps = []
    for c in range(NCORES):
        pa = patches[:, :, 2 * c:2 * c + 2, :, :]        # (N, C, 2, P, F2)
        a2 = pa.transpose(1, 4, 2, 3, 0)                 # (C, F2, 2, P, N)
        a3 = (a2.reshape(K, POS, N)
                .reshape(KT, 128, POS, N)
                .transpose(1, 2, 0, 3)                   # (128, POS, KT, N)
                .reshape(128, POS * KT * N))
        xr_c = np.ascontiguousarray(a3).astype(FP8 if X_FP8 else BF16)

        wb = ws[:, :, 2 * c:2 * c + 2, :, :]             # (O, C, 2, P, F2)
        b2 = wb.transpose(1, 4, 2, 3, 0)                 # (C, F2, 2, P, O)
        b3 = (b2.reshape(K, POS, O)
                .reshape(KT, 128, POS, O)
                .transpose(1, 2, 0, 3)                   # (128, POS, KT, O)
                .reshape(128, POS * KT * O))
        wr_c = np.ascontiguousarray(b3).astype(FP8)

        in_maps.append({"xr": xr_c, "wr": wr_c})
    return in_maps


def kernel(x: np.ndarray, w: np.ndarray) -> np.ndarray:
    from concourse.bass_utils import run_bass_kernel_spmd

    nc = _build_program()
    in_maps = _prep_inputs(np.asarray(x), np.asarray(w))

    res = run_bass_kernel_spmd(nc, in_maps, core_ids=list(range(NCORES)))
    _cache["last_results"] = res

    y = np.empty((N, O, P, P), dtype=np.float32)
    for c in range(NCORES):
        y[:, :, 2 * c:2 * c + 2, :] = decode_core(res.results[c]["yr"])
    return y


def decode_core(yr: np.ndarray) -> np.ndarray:
    """(128, PAIRS*O) core output -> (N, O, PROWS_PER_CORE, P) slice.

    yr[r, pair*O + o] with r = (pos%2)*64 + n, pos = pair*2 + (pos%2) and
    pos = pl*P + q.
    """
    yrr = (yr.astype(np.float32)
             .reshape(2, N, PAIRS, O)          # (ab, n, pair, o)
             .transpose(2, 0, 1, 3)            # (pair, ab, n, o)
             .reshape(POS, N, O))              # (pos, n, o)
    return yrr.reshape(PROWS_PER_CORE, P, N, O).transpose(2, 3, 0, 1)


# revision 9
# speedup vs baseline: 1.1546x; 1.1546x over previous
"""Locally-connected 2D block layer (LocBlock2dNT) on 8 Trainium2 NeuronCores.

Problem: x (64,64,64,64) f32, w (256,64,16,16,16) f32.
  patches = unfold(x) -> (N,C,P,P,f2);  y = relu(einsum('ncpqf,ocpqf->nopq', patches, w) / 32)

Strategy (v3, fp8 weights + fp8 activations, balanced DMA rings):
  - Shard over patch ROWS p (16 rows, 2 per core). Both x and w shard cleanly
    along p: zero replication.
  - Host-side (free): unfold + transpose into a K-major layout. w and x are
    cast to fp8 e3m4 (4 mantissa bits; ~1.9% end-to-end rel err vs the 2e-2
    budget) WITHOUT the 1/32 scale (values must stay in e3m4's normal range);
    the scale is applied in the epilogue: relu(ps/32) = max(ps*1/32, 0) as a
    two-op DVE tensor_scalar.
  - DMA per core: w 8.4 MB + x 2.1 MB in, y 1.05 MB out. Inputs stream in 8
    chunks alternating across the two HWDGE rings (sync/scalar) by parity so
    both rings carry equal bytes; outputs ride both rings after all inputs
    (never blocking an input DMA behind a compute dependency).
  - Per core: 32 positions, each an [M=64 batch] x [K=1024] x [N=256 outch]
    matmul (x stationary, w moving, both fp8e3). Positions packed
    two-at-a-time into the 128-wide PE column dim (pos A -> PSUM partitions
    0:64, pos B -> 64:128) so the two N=256 matmul streams run concurrently.
  - Epilogue: scale+relu on DVE, PSUM -> SBUF -> DRAM.
"""

import numpy as np
import ml_dtypes

N = 64          # batch
C = 64          # in channels
P = 16          # patches per side
F = 4           # filter side
F2 = F * F      # 16
O = 256         # out channels
K = C * F2      # 1024 contraction
NCORES = 8
PROWS_PER_CORE = P // NCORES      # 2
POS = PROWS_PER_CORE * P          # 32 positions per core
PAIRS = POS // 2                  # 16
KT = K // 128                     # 8 k-tiles
SCALE = 1.0 / np.sqrt(np.float32(F2 * C))   # == 1/32 exactly

BF16 = ml_dtypes.bfloat16
FP8 = ml_dtypes.float8_e3m4

X_FP8 = True                # x in fp8 e3m4 (else bf16 with scale folded in)

# chunk sizes (positions per streaming chunk): big chunks first for DMA
# efficiency (16KB partition lines), small last chunk for a short compute
# tail. Alternating ring assignment of w/x/out per chunk index gives both
# HWDGE rings exactly (w+x+y)/2 bytes.
CHUNKS = [8, 8, 4, 4, 4, 4]
assert sum(CHUNKS) == POS

_cache = {}


def _build_program():
    """Build + compile the (SPMD, shared) Bass program once per process."""
    if "nc" in _cache:
        return _cache["nc"]

    import concourse.bacc as bacc
    import concourse.mybir as mybir
    import concourse.tile as tile

    xdt = mybir.dt.float8e3 if X_FP8 else mybir.dt.bfloat16

    nc = bacc.Bacc(
        "TRN2", target_bir_lowering=False, debug=False, num_devices=NCORES
    )
    xr = nc.dram_tensor("xr", (128, POS * KT * N), xdt,
                        kind="ExternalInput").ap()
    wr = nc.dram_tensor("wr", (128, POS * KT * O), mybir.dt.float8e3,
                        kind="ExternalInput").ap()
    # yr[r, pair*256 + o], r = (pos%2)*64 + n
    yr = nc.dram_tensor("yr", (128, PAIRS * O), mybir.dt.bfloat16,
                        kind="ExternalOutput").ap()

    starts = [sum(CHUNKS[:i]) for i in range(len(CHUNKS))]   # start positions

    with tile.TileContext(nc) as tc:
        QS = [nc.sync, nc.scalar]   # the two HWDGE rings
        with (
            tc.tile_pool(name="xpool", bufs=1) as xpool,
            tc.tile_pool(name="wpool", bufs=1) as wpool,
            tc.tile_pool(name="pspool", bufs=4, space="PSUM") as pspool,
            tc.tile_pool(name="opool", bufs=3) as opool,
        ):
            # whole x + w resident in SBUF, streamed in chunk-sized slices in
            # consumption order, alternating rings so both carry equal bytes.
            xall = xpool.tile([128, POS * KT * N], xdt)
            wall = wpool.tile([128, POS * KT * O], mybir.dt.float8e3)
            for c, (p0, np_) in enumerate(zip(starts, CHUNKS)):
                w0, w1 = p0 * KT * O, (p0 + np_) * KT * O
                x0, x1 = p0 * KT * N, (p0 + np_) * KT * N
                QS[c % 2].dma_start(out=wall[:, w0:w1], in_=wr[:, w0:w1])
                QS[(c + 1) % 2].dma_start(out=xall[:, x0:x1], in_=xr[:, x0:x1])

            for chunk, (p0, np_) in enumerate(zip(starts, CHUNKS)):
                ot = opool.tile([128, (np_ // 2) * O], mybir.dt.bfloat16)
                for jp in range(np_ // 2):     # position pairs in chunk
                    pos_a = p0 + 2 * jp
                    pos_b = pos_a + 1
                    # two PSUM banks so the two concurrent accumulation
                    # groups never share a zero region
                    psa = pspool.tile([N, O], mybir.dt.float32)
                    psb_full = pspool.tile([128, O], mybir.dt.float32)
                    psb = psb_full[N:2 * N, :]
                    for k in range(KT):
                        xa = xall[:, pos_a * KT * N + k * N:
                                     pos_a * KT * N + k * N + N]
                        xb = xall[:, pos_b * KT * N + k * N:
                                     pos_b * KT * N + k * N + N]
                        wa = wall[:, pos_a * KT * O + k * O:
                                     pos_a * KT * O + k * O + O]
                        wb = wall[:, pos_b * KT * O + k * O:
                                     pos_b * KT * O + k * O + O]
                        # A -> array col group 0:64, B -> 64:128; the two
                        # matmul streams run concurrently
                        nc.tensor.matmul(psa, xa, wa,
                                         start=(k == 0), stop=(k == KT - 1))
                        nc.tensor.matmul(psb, xb, wb,
                                         start=(k == 0), stop=(k == KT - 1))
                    oc = jp * O
                    if X_FP8:
                        # relu(ps/32) == max(ps * 1/32, 0)
                        nc.vector.tensor_scalar(
                            ot[0:N, oc:oc + O], psa, float(SCALE), 0.0,
                            mybir.AluOpType.mult, mybir.AluOpType.max)
                        nc.vector.tensor_scalar(
                            ot[N:2 * N, oc:oc + O], psb, float(SCALE), 0.0,
                            mybir.AluOpType.mult, mybir.AluOpType.max)
                    else:
                        nc.vector.tensor_scalar_max(ot[0:N, oc:oc + O],
                                                    psa, 0.0)
                        nc.vector.tensor_scalar_max(ot[N:2 * N, oc:oc + O],
                                                    psb, 0.0)
                # one output DMA per chunk, alternating rings, issued after
                # all input DMAs in each ring's FIFO order
                pair0 = p0 // 2
                QS[chunk % 2].dma_start(
                    out=yr[:, pair0 * O:(pair0 + np_ // 2) * O], in_=ot)

    nc.compile()
    _cache["nc"] = nc
    return nc


def _prep_inputs(x: np.ndarray, w: np.ndarray):
    """Host-side shard + layout + cast. Returns in_maps for 8 cores.

    Layouts per core (core c owns patch rows 2c, 2c+1; pos = pl*16 + q):
      xr[p128, pos, k, n] = patches[n, ch, 2c+pl, q, f],  K = k*128+p128 = ch*16+f
      wr[p128, pos, k, o] = w[o, ch, 2c+pl, q, f]
      yr row = pair*128 + (pos%2)*64 + n
    Both fp8 e3m4 unscaled; the 1/32 scale is applied on-chip in the epilogue.
    """
    # unfold: (N,C,P,f,P,f) -> (N,C,P,P,f,f) -> (N,C,P,P,f2)
    patches = np.ascontiguousarray(
        x.reshape(N, C, P, F, P, F).transpose(0, 1, 2, 4, 3, 5)
    ).reshape(N, C, P, P, F2)
    if not X_FP8:
        patches = patches * SCALE
    ws = w.astype(np.float32)

    in_maps = []
    for c in range(NCORES):
        pa = patches[:, :, 2 * c:2 * c + 2, :, :]        # (N, C, 2, P, F2)
        a2 = pa.transpose(1, 4, 2, 3, 0)                 # (C, F2, 2, P, N)
        a3 = (a2.reshape(K, POS, N)
                .reshape(KT, 128, POS, N)
                .transpose(1, 2, 0, 3)                   # (128, POS, KT, N)
                .reshape(128, POS * KT * N))
        xr_c = np.ascontiguousarray(a3).astype(FP8 if X_FP8 else BF16)

        wb = ws[:, :, 2 * c:2 * c + 2, :, :]             # (O, C, 2, P, F2)
        b2 = wb.transpose(1, 4, 2, 3, 0)                 # (C, F2, 2, P, O)
        b3 = (b2.reshape(K, POS, O)
                .reshape(KT, 128, POS, O)
                .transpose(1, 2, 0, 3)                   # (128, POS, KT, O)
                .reshape(128, POS * KT * O))
        wr_c = np.ascontiguousarray(b3).astype(FP8)

        in_maps.append({"xr": xr_c, "wr": wr_c})
    return in_maps


def kernel(x: np.ndarray, w: np.ndarray) -> np.ndarray:
    from concourse.bass_utils import run_bass_kernel_spmd

    nc = _build_program()
    in_maps = _prep_inputs(np.asarray(x), np.asarray(w))

    res = run_bass_kernel_spmd(nc, in_maps, core_ids=list(range(NCORES)))
    _cache["last_results"] = res

    y = np.empty((N, O, P, P), dtype=np.float32)
    for c in range(NCORES):
        y[:, :, 2 * c:2 * c + 2, :] = decode_core(res.results[c]["yr"])
    return y


def decode_core(yr: np.ndarray) -> np.ndarray:
    """(128, PAIRS*O) core output -> (N, O, PROWS_PER_CORE, P) slice.

    yr[r, pair*O + o] with r = (pos%2)*64 + n, pos = pair*2 + (pos%2) and
    pos = pl*P + q.
    """
    yrr = (yr.astype(np.float32)
             .reshape(2, N, PAIRS, O)          # (ab, n, pair, o)
             .transpose(2, 0, 1, 3)            # (pair, ab, n, o)
             .reshape(POS, N, O))              # (pos, n, o)
    return yrr.reshape(PROWS_PER_CORE, P, N, O).transpose(2, 3, 0, 1)


# revision 11
# speedup vs baseline: 1.3586x; 1.1767x over previous
"""Locally-connected 2D block layer (LocBlock2dNT) on 8 Trainium2 NeuronCores.

Problem: x (64,64,64,64) f32, w (256,64,16,16,16) f32.
  patches = unfold(x) -> (N,C,P,P,f2);  y = relu(einsum('ncpqf,ocpqf->nopq', patches, w) / 32)

Strategy (v3, fp8 weights + fp8 activations, balanced DMA rings):
  - Shard over patch ROWS p (16 rows, 2 per core). Both x and w shard cleanly
    along p: zero replication.
  - Host-side (free): unfold + transpose into a K-major layout. w and x are
    cast to fp8 e3m4 (4 mantissa bits; ~1.9% end-to-end rel err vs the 2e-2
    budget) WITHOUT the 1/32 scale (values must stay in e3m4's normal range);
    the scale is applied in the epilogue: relu(ps/32) = max(ps*1/32, 0) as a
    two-op DVE tensor_scalar.
  - DMA per core: w 8.4 MB + x 2.1 MB in, y 1.05 MB out. Inputs stream in 8
    chunks alternating across the two HWDGE rings (sync/scalar) by parity so
    both rings carry equal bytes; outputs ride both rings after all inputs
    (never blocking an input DMA behind a compute dependency).
  - Per core: 32 positions, each an [M=64 batch] x [K=1024] x [N=256 outch]
    matmul (x stationary, w moving, both fp8e3). Positions packed
    two-at-a-time into the 128-wide PE column dim (pos A -> PSUM partitions
    0:64, pos B -> 64:128) so the two N=256 matmul streams run concurrently.
  - Epilogue: scale+relu on DVE, PSUM -> SBUF -> DRAM.
"""

import numpy as np
import ml_dtypes

N = 64          # batch
C = 64          # in channels
P = 16          # patches per side
F = 4           # filter side
F2 = F * F      # 16
O = 256         # out channels
K = C * F2      # 1024 contraction
NCORES = 8
PROWS_PER_CORE = P // NCORES      # 2
POS = PROWS_PER_CORE * P          # 32 positions per core
PAIRS = POS // 2                  # 16
KT = K // 128                     # 8 k-tiles
SCALE = 1.0 / np.sqrt(np.float32(F2 * C))   # == 1/32 exactly

BF16 = ml_dtypes.bfloat16
FP8 = ml_dtypes.float8_e3m4

X_FP8 = True                # x in fp8 e3m4 (else bf16 with scale folded in)

# chunk sizes (positions per streaming chunk): big chunks first for DMA
# efficiency (16KB partition lines), small last chunk for a short compute
# tail. Alternating ring assignment of w/x/out per chunk index gives both
# HWDGE rings exactly (w+x+y)/2 bytes.
CHUNKS = [8, 8, 4, 4, 4, 4]
assert sum(CHUNKS) == POS

_cache = {}


def _build_program():
    """Build + compile the (SPMD, shared) Bass program once per process."""
    if "nc" in _cache:
        return _cache["nc"]

    import concourse.bacc as bacc
    import concourse.mybir as mybir
    import concourse.tile as tile

    xdt = mybir.dt.float8e3 if X_FP8 else mybir.dt.bfloat16

    nc = bacc.Bacc(
        "TRN2", target_bir_lowering=False, debug=False, num_devices=NCORES
    )
    xr = nc.dram_tensor("xr", (128, POS * KT * N), xdt,
                        kind="ExternalInput").ap()
    wr = nc.dram_tensor("wr", (128, POS * KT * O), mybir.dt.float8e3,
                        kind="ExternalInput").ap()
    # yr[r, pair*256 + o], r = (pos%2)*64 + n
    yr = nc.dram_tensor("yr", (128, PAIRS * O), mybir.dt.bfloat16,
                        kind="ExternalOutput").ap()

    starts = [sum(CHUNKS[:i]) for i in range(len(CHUNKS))]   # start positions

    with tile.TileContext(nc) as tc:
        QS = [nc.sync, nc.scalar]   # the two HWDGE rings
        with (
            tc.tile_pool(name="xpool", bufs=1) as xpool,
            tc.tile_pool(name="wpool", bufs=1) as wpool,
            tc.tile_pool(name="pspool", bufs=4, space="PSUM") as pspool,
            tc.tile_pool(name="opool", bufs=len(CHUNKS)) as opool,
        ):
            # whole x + w resident in SBUF, streamed in chunk-sized slices in
            # consumption order, alternating rings so both carry equal bytes.
            xall = xpool.tile([128, POS * KT * N], xdt)
            wall = wpool.tile([128, POS * KT * O], mybir.dt.float8e3)

            def load_chunk(c):
                p0, np_ = starts[c], CHUNKS[c]
                w0, w1 = p0 * KT * O, (p0 + np_) * KT * O
                x0, x1 = p0 * KT * N, (p0 + np_) * KT * N
                QS[c % 2].dma_start(out=wall[:, w0:w1], in_=wr[:, w0:w1])
                QS[(c + 1) % 2].dma_start(out=xall[:, x0:x1], in_=xr[:, x0:x1])

            # two-chunk DMA lookahead; out DMAs interleave between input
            # chunks on the rings (by the time a ring's sequencer reaches
            # out_c it has drained inputs through c+2, so the compute-c
            # dependency is long satisfied and never blocks an input).
            load_chunk(0)
            load_chunk(1)
            for chunk, (p0, np_) in enumerate(zip(starts, CHUNKS)):
                if chunk + 2 < len(CHUNKS):
                    load_chunk(chunk + 2)
                ot = opool.tile([128, (np_ // 2) * O], mybir.dt.bfloat16)
                for jp in range(np_ // 2):     # position pairs in chunk
                    pos_a = p0 + 2 * jp
                    pos_b = pos_a + 1
                    # two PSUM banks so the two concurrent accumulation
                    # groups never share a zero region
                    psa = pspool.tile([N, O], mybir.dt.float32)
                    psb_full = pspool.tile([128, O], mybir.dt.float32)
                    psb = psb_full[N:2 * N, :]
                    for k in range(KT):
                        xa = xall[:, pos_a * KT * N + k * N:
                                     pos_a * KT * N + k * N + N]
                        xb = xall[:, pos_b * KT * N + k * N:
                                     pos_b * KT * N + k * N + N]
                        wa = wall[:, pos_a * KT * O + k * O:
                                     pos_a * KT * O + k * O + O]
                        wb = wall[:, pos_b * KT * O + k * O:
                                     pos_b * KT * O + k * O + O]
                        # A -> array col group 0:64, B -> 64:128; the two
                        # matmul streams run concurrently
                        nc.tensor.matmul(psa, xa, wa,
                                         start=(k == 0), stop=(k == KT - 1))
                        nc.tensor.matmul(psb, xb, wb,
                                         start=(k == 0), stop=(k == KT - 1))
                    oc = jp * O
                    if X_FP8:
                        # relu(ps/32) == max(ps * 1/32, 0)
                        nc.vector.tensor_scalar(
                            ot[0:N, oc:oc + O], psa, float(SCALE), 0.0,
                            mybir.AluOpType.mult, mybir.AluOpType.max)
                        nc.vector.tensor_scalar(
                            ot[N:2 * N, oc:oc + O], psb, float(SCALE), 0.0,
                            mybir.AluOpType.mult, mybir.AluOpType.max)
                    else:
                        nc.vector.tensor_scalar_max(ot[0:N, oc:oc + O],
                                                    psa, 0.0)
                        nc.vector.tensor_scalar_max(ot[N:2 * N, oc:oc + O],
                                                    psb, 0.0)
                # one output DMA per chunk, alternating rings, issued after
                # all input DMAs in each ring's FIFO order
                pair0 = p0 // 2
                QS[chunk % 2].dma_start(
                    out=yr[:, pair0 * O:(pair0 + np_ // 2) * O], in_=ot)

    nc.compile()
    _cache["nc"] = nc
    return nc


def _prep_inputs(x: np.ndarray, w: np.ndarray):
    """Host-side shard + layout + cast. Returns in_maps for 8 cores.

    Layouts per core (core c owns patch rows 2c, 2c+1; pos = pl*16 + q):
      xr[p128, pos, k, n] = patches[n, ch, 2c+pl, q, f],  K = k*128+p128 = ch*16+f
      wr[p128, pos, k, o] = w[o, ch, 2c+pl, q, f]
      yr row = pair*128 + (pos%2)*64 + n
    Both fp8 e3m4 unscaled; the 1/32 scale is applied on-chip in the epilogue.
    """
    # unfold: (N,C,P,f,P,f) -> (N,C,P,P,f,f) -> (N,C,P,P,f2)
    patches = np.ascontiguousarray(
        x.reshape(N, C, P, F, P, F).transpose(0, 1, 2, 4, 3, 5)
    ).reshape(N, C, P, P, F2)
    if not X_FP8:
        patches = patches * SCALE
    ws = w.astype(np.float32)

    in_maps = []
    for c in range(NCORES):
        pa = patches[:, :, 2 * c:2 * c + 2, :, :]        # (N, C, 2, P, F2)
        a2 = pa.transpose(1, 4, 2, 3, 0)                 # (C, F2, 2, P, N)
        a3 = (a2.reshape(K, POS, N)
                .reshape(KT, 128, POS, N)
                .transpose(1, 2, 0, 3)                   # (128, POS, KT, N)
                .reshape(128, POS * KT * N))
        xr_c = np.ascontiguousarray(a3).astype(FP8 if X_FP8 else BF16)

        wb = ws[:, :, 2 * c:2 * c + 2, :, :]             # (O, C, 2, P, F2)
        b2 = wb.transpose(1, 4, 2, 3, 0)                 # (C, F2, 2, P, O)
        b3 = (b2.reshape(K, POS, O)
                .reshape(KT, 128, POS, O)
                .transpose(1, 2, 0, 3)                   # (128, POS, KT, O)
                .reshape(128, POS * KT * O))
        wr_c = np.ascontiguousarray(b3).astype(FP8)

        in_maps.append({"xr": xr_c, "wr": wr_c})
    return in_maps


def kernel(x: np.ndarray, w: np.ndarray) -> np.ndarray:
    from concourse.bass_utils import run_bass_kernel_spmd

    nc = _build_program()
    in_maps = _prep_inputs(np.asarray(x), np.asarray(w))

    res = run_bass_kernel_spmd(nc, in_maps, core_ids=list(range(NCORES)))
    _cache["last_results"] = res

    y = np.empty((N, O, P, P), dtype=np.float32)
    for c in range(NCORES):
        y[:, :, 2 * c:2 * c + 2, :] = decode_core(res.results[c]["yr"])
    return y


def decode_core(yr: np.ndarray) -> np.ndarray:
    """(128, PAIRS*O) core output -> (N, O, PROWS_PER_CORE, P) slice.

    yr[r, pair*O + o] with r = (pos%2)*64 + n, pos = pair*2 + (pos%2) and
    pos = pl*P + q.
    """
    yrr = (yr.astype(np.float32)
             .reshape(2, N, PAIRS, O)          # (ab, n, pair, o)
             .transpose(2, 0, 1, 3)            # (pair, ab, n, o)
             .reshape(POS, N, O))              # (pos, n, o)
    return yrr.reshape(PROWS_PER_CORE, P, N, O).transpose(2, 3, 0, 1)
